# revision 12
# baseline (speedup 1.0000x reference)
"""Trainium2 Bass kernel for the AdapterController hard-routing MoE adapter.

Reference computation (per router m in [0,4), batch b in [0,16)):
    e = expert_index[m, b]
    z = x[b] @ down_w[m, e] + down_b[m, e]      # [512, 256]
    z = z * sigmoid(z)                          # swish
    u = z @ up_w[m, e]                          # [512, 1024]
    out[m, b] = u

Strategy: data-parallel over the batch axis (2 batches per core, 8 cores).
The expert gather is part of input sharding: each core receives exactly the
(m, b)-selected weight matrices, packed on the host into the SBUF partition
layout so every DMA is fully contiguous.

On-chip per (m, b) pair:
    zT[d, s] = sum_c Wd[c, d] * xT[c, s]        (16 matmuls N=512, K=128)
    z = silu(zT + bd)                           (ACT engine, PSUM -> SBUF)
    u[s, c] = sum_d zT[d, s].T @ Wu[d, c]       (16 matmuls N=512)

Schedule notes (v2): the matmul stream runs at the warm-PE roofline
(~216 ns per N=512 matmul, 54.6 us total), so the schedule optimizes the
edges:
  - three DMA rings are used: sync (SP HWDGE), scalar (ACT HWDGE), and
    gpsimd (Pool SWDGE).  Head: sync carries wd0 + x(b0) (the stream-start
    critical path; the scalar ring is handicapped ~1.3us by the auto
    act-table load), pool carries bias + wu0 + x(b1), scalar carries
    wd1/wu1.  Steady state: wd(q) on scalar, wu(q) on sync, outs split
    a0/a2->pool, a1->sync, a3->scalar so no ring needs more than ~40% of
    the 360 GB/s DMA-engine pool.
  - the bias transfer is padded to 576B rows (rows below the 512B SDMA
    line-rate minimum degrade into read-modify-write descriptors);
  - the PE HAM clock gate needs ~3.4us of *uninterrupted* matmul activity
    to lift the 1.2 GHz cold throttle, and an idle gap restarts the ramp:
    the warm-up burst of N=128 matmuls is sized so the PE never idles
    between warm-up end and first-data arrival (~11.7us);
  - the first pair runs its down-projection k-outer so both PSUM groups
    chase each arriving x chunk;
  - the last pair reorders its up-projection so a2 completes first and the
    final a3 block is h-major; final copies alternate vector/scalar and the
    final outs drain on three rings in parallel to shorten the tail.
"""

import numpy as np

M, B, S, C, D = 4, 16, 512, 1024, 256
N_CORES = 8
B_LOC = B // N_CORES  # batches per core
KC = C // 128         # 8 c-chunks
KD = D // 128         # 2 d-chunks
NPAIR = M * B_LOC     # 8 (m, b) pairs per core
WARM = 52             # warm-up matmuls (N=128) bridging until first data

_cache = {}
last_results = None  # BassKernelResults of the most recent run (for test.py)


def _build():
    from contextlib import ExitStack

    import concourse.mybir as mybir
    import concourse.tile as tile
    from concourse import bacc
    f32 = mybir.dt.float32
    bf16 = mybir.dt.bfloat16
    mm_dt = bf16
    out_dt = bf16

    nc = bacc.Bacc("TRN2", target_bir_lowering=False, debug=False,
                   num_devices=N_CORES)
    # xtp[b, half][p, k*512 + s] = x[b, s, 128*(4*half + k) + p]
    xtp = nc.dram_tensor("xtp", [B_LOC, 2, 128, KC * S // 2], bf16,
                         kind="ExternalInput").ap()
    # wdp[m, b][p, k*256 + d] = down_w_gathered[m, b, 128k + p, d]
    wdp = nc.dram_tensor("wdp", [M, B_LOC, 128, KC * D], bf16,
                         kind="ExternalInput").ap()
    # bdp[p, (m*B_LOC+b)*2 + j] = down_b_gathered[m, b, 128j + p]; padded to
    # 144 f32 columns so each DMA row is 576B (>= the 512B SDMA line-rate
    # minimum)
    bdp = nc.dram_tensor("bdp", [128, 144], f32, kind="ExternalInput").ap()
    # wup[m, b][p, j*1024 + c] = up_w_gathered[m, b, 128j + p, c]
    wup = nc.dram_tensor("wup", [M, B_LOC, 128, KD * C], bf16,
                         kind="ExternalInput").ap()
    out = nc.dram_tensor("out", [M, B_LOC, S, C], out_dt,
                         kind="ExternalOutput").ap()

    silu = mybir.ActivationFunctionType.Silu
    copy_fn = mybir.ActivationFunctionType.Copy

    with tile.TileContext(nc) as tc, ExitStack() as ctx:
        const = ctx.enter_context(tc.tile_pool(name="const", bufs=1))
        xpool = ctx.enter_context(tc.tile_pool(name="xpool", bufs=4))
        wdpool = ctx.enter_context(tc.tile_pool(name="wdpool", bufs=4))
        wupool = ctx.enter_context(tc.tile_pool(name="wupool", bufs=4))
        zpool = ctx.enter_context(tc.tile_pool(name="zpool", bufs=2))
        upool = ctx.enter_context(tc.tile_pool(name="upool", bufs=12))
        pszp = ctx.enter_context(tc.tile_pool(name="pszp", bufs=2, space="PSUM"))
        psup = ctx.enter_context(tc.tile_pool(name="psup", bufs=3, space="PSUM"))

        bd_sb = const.tile([128, 144], f32)

        # PE warm-up: the HAM clock gate needs ~3.4us of uninterrupted PE
        # activity to lift the 1.2GHz cold throttle, and an idle gap restarts
        # the ramp; the burst is sized to bridge from program start until the
        # first weights/x land (~12us)
        warm_src = const.tile([128, 128], mm_dt)
        nc.gpsimd.memset(warm_src[:], 0)
        warm_ps = pszp.tile([128, 128], f32, tag="psz", name="warm_ps")
        for _ in range(WARM):
            nc.tensor.matmul(warm_ps[:], warm_src[:], warm_src[:],
                             start=True, stop=True)

        # Each DMA ring drains FIFO in trigger order, so per-ring emission
        # order pins byte-arrival order. (No dep-chaining between DMAs: a dep
        # on a DMA instruction waits for its data semaphore, which would
        # serialize transfer-after-completion.)
        xh = {b: [xpool.tile([128, KC * S // 2], mm_dt, tag="xt",
                             name=f"xt_{b}_{h}") for h in range(2)]
              for b in range(B_LOC)}
        wd_t = {p: wdpool.tile([128, KC * D], mm_dt, tag="wd",
                               name=f"wd{p}")
                for p in range(2)}
        wu_t = {p: wupool.tile([128, KD * C], mm_dt, tag="wu", name=f"wu{p}")
                for p in range(2)}

        # Head fill.  HW model: each HWDGE engine (sync=SP, scalar=ACT) owns
        # 4 hardware queues; every un-blocked dma_start costs ~0.6us on its
        # sequencer AND ~0.63us on the single shared HWDGE descriptor
        # generator, then all ACTIVE queues share the 16 DMA engines
        # (~360 GB/s) round-robin.  Transfers on one engine do NOT
        # serialize; arrival order is set by cfg-stagger + fair sharing.
        # So: few transfers, critical ones (x00 on sync, wd0 on scalar)
        # first into the generator; everything later paced behind blocking
        # instructions so it can't steal head bandwidth.
        nc.sync.dma_start(xh[0][0][:], xtp[0, 0])    # x00
        nc.sync.dma_start(xh[0][1][:], xtp[0, 1])    # x01
        nc.scalar.dma_start(wd_t[0][:], wdp[0, 0])   # wd0
        nc.scalar.dma_start(wu_t[0][:, :1024], wup[0, 0][:, :1024])  # wu0-j0
        nc.scalar.dma_start(wu_t[0][:, 1024:], wup[0, 0][:, 1024:])  # wu0-j1
        nc.scalar.dma_start(bd_sb[:], bdp[:])        # bias (576B rows)
        # dummy silu: forces the ~1.3us act-table load HERE (scalar seq),
        # which also delays the wd1/wu1/wu2 cfgs below past the head burst
        dummy_sb = const.tile([128, 8], mm_dt)
        nc.scalar.activation(dummy_sb[:], warm_src[:, 0:8], silu)
        nc.scalar.dma_start(wd_t[1][:], wdp[1, 0])   # wd1
        nc.scalar.dma_start(wu_t[1][:], wup[1, 0])   # wu1
        wu_t[2] = wupool.tile([128, KD * C], mm_dt, tag="wu", name="wu2")
        nc.scalar.dma_start(wu_t[2][:], wup[2 % M, 2 // M])   # wu2
        # x(b1) rides the pool ring, held back by a WAW dep on its first
        # columns (written from pair 0's first out tile) so it can't start
        # before ~T0+4us no matter how the scheduler orders the pool stream.

        for p in range(NPAIR):
            m, b = p % M, p // M
            q = p + 2
            if q < NPAIR:
                mq, bq = q % M, q // M
                wd_t[q] = wdpool.tile([128, KC * D], mm_dt, tag="wd",
                                      name=f"wd{q}")
            q3 = p + 3
            if 3 <= q3 < NPAIR:
                wu_t[q3] = wupool.tile([128, KD * C], mm_dt, tag="wu",
                                       name=f"wu{q3}")

            wd_sb, wu_sb, xb = wd_t[p], wu_t[p], xh[b]
            z_sb = zpool.tile([128, KD, S], mm_dt)
            if p == 0:
                # k-outer for the first pair: both PSUM groups consume each
                # arriving x chunk, halving the early x-consumption rate so
                # the matmuls bridge the x(b0) half-1 delivery
                psz_j = [pszp.tile([128, S], f32, tag="psz", name=f"psz{j}")
                         for j in range(KD)]
                for k in range(KC):
                    for j in range(KD):
                        nc.tensor.matmul(
                            psz_j[j][:],
                            wd_sb[:, k * 256 + j * 128:
                                  k * 256 + j * 128 + 128],
                            xb[k // 4][:, (k % 4) * S: (k % 4 + 1) * S],
                            start=(k == 0), stop=(k == KC - 1),
                        )
                for j in range(KD):
                    col = (m * B_LOC + b) * KD + j
                    nc.scalar.activation(z_sb[:, j, :], psz_j[j][:], silu,
                                         bias=bd_sb[:, col: col + 1])
                    if j == 0 and q < NPAIR:
                        # wd(q) cfg is paced behind silu j0 on the scalar
                        # sequencer: it can't fire before ~T0+2us, keeping
                        # the head queues clear for the critical bytes
                        nc.scalar.dma_start(wd_t[q][:], wdp[q % M, q // M])
            else:
                for j in range(KD):
                    psz = pszp.tile([128, S], f32)
                    for k in range(KC):
                        nc.tensor.matmul(
                            psz[:],
                            wd_sb[:, k * 256 + j * 128:
                                  k * 256 + j * 128 + 128],
                            xb[k // 4][:, (k % 4) * S: (k % 4 + 1) * S],
                            start=(k == 0), stop=(k == KC - 1),
                        )
                    col = (m * B_LOC + b) * KD + j
                    nc.scalar.activation(z_sb[:, j, :], psz[:], silu,
                                         bias=bd_sb[:, col: col + 1])
                    if j == 0 and q < NPAIR:
                        nc.scalar.dma_start(wd_t[q][:], wdp[q % M, q // M])

            # one 2-bank PSUM tile per a holds a full [128, 1024] u row;
            # groups are interleaved so j=1 matmuls trail the j=1 silu by a
            # couple of matmul slots (no PE stall).
            last = p == NPAIR - 1
            if not last:
                order = ((0, 0), (0, 1), (1, 0), (1, 1),
                         (0, 2), (0, 3), (1, 2), (1, 3))
                psu_by_a = {}
                for j, a in order:
                    if j == 0:
                        psu_by_a[a] = psup.tile([128, C], f32, tag="psu",
                                                name=f"psu_{p}_{a}")
                    psu = psu_by_a[a]
                    for h in range(2):
                        nc.tensor.matmul(
                            psu[:, h * 512: (h + 1) * 512],
                            z_sb[:, j, a * 128: (a + 1) * 128],
                            wu_sb[:, j * 1024 + h * 512:
                                  j * 1024 + h * 512 + 512],
                            start=(j == 0), stop=(j == KD - 1),
                            skip_group_check=True,
                        )
                    if j == KD - 1:
                        u_sb = upool.tile([128, C], out_dt, tag="u")
                        orow = out[m, b, a * 128:(a + 1) * 128, :]
                        if a % 2 == 0:
                            nc.vector.tensor_copy(u_sb[:], psu[:])
                        else:
                            nc.scalar.activation(u_sb[:], psu[:], copy_fn)
                        # outs: a0/a2 on pool queues, a1 on sync, a3 on
                        # scalar -- spreads output traffic so weight
                        # transfers never queue behind it
                        if a in (0, 2):
                            nc.gpsimd.dma_start(orow, u_sb[:])
                        elif a == 1:
                            nc.sync.dma_start(orow, u_sb[:])
                            if 3 <= p + 3 < NPAIR:
                                # wu(p+3) cfg paced behind the a1 out
                                # trigger on the sync sequencer (~T0+5+6.8p)
                                nc.sync.dma_start(wu_t[p + 3][:],
                                                 wup[(p + 3) % M,
                                                     (p + 3) // M])
                        else:
                            nc.scalar.dma_start(orow, u_sb[:])
                        if p == 0 and a == 0:
                            # x(b1): a WAW dep on its first columns (written
                            # from pair 0's first out tile) holds the
                            # transfer back until ~T0+4us regardless of how
                            # the scheduler orders the pool stream
                            nc.gpsimd.tensor_copy(xh[1][0][:, 0:4],
                                                  u_sb[:, 0:4])
                            nc.gpsimd.dma_start(xh[1][0][:], xtp[1, 0])
                            nc.gpsimd.tensor_copy(xh[1][1][:, 0:4],
                                                  u_sb[:, 4:8])
                            nc.gpsimd.dma_start(xh[1][1][:], xtp[1, 1])
            else:
                # Final pair: a2 completes first, then a0, a1; a3 runs
                # h-major so its first half can be copied + DMA'd while the
                # second half's matmuls run.  Copies split vector/scalar so
                # neither engine's chain extends past the last matmul by
                # more than one half-copy; out cfgs spread over sync (3) and
                # pool (3) so no sequencer serializes more than ~3 cfgs.
                order = ((0, 2), (0, 0), (1, 2), (1, 0), (0, 1), (1, 1))
                psu_by_a = {}
                u_by_a = {}
                for j, a in order:
                    if j == 0:
                        psu_by_a[a] = psup.tile([128, C], f32, tag="psu",
                                                name=f"psu_{p}_{a}")
                    psu = psu_by_a[a]
                    for h in range(2):
                        nc.tensor.matmul(
                            psu[:, h * 512: (h + 1) * 512],
                            z_sb[:, j, a * 128: (a + 1) * 128],
                            wu_sb[:, j * 1024 + h * 512:
                                  j * 1024 + h * 512 + 512],
                            start=(j == 0), stop=(j == KD - 1),
                            skip_group_check=True,
                        )
                    if j == KD - 1:
                        u_sb = upool.tile([128, C], out_dt, tag="u",
                                          name=f"u_last_{a}")
                        u_by_a[a] = u_sb
                        orow = out[m, b, a * 128:(a + 1) * 128, :]
                        if a == 2:
                            nc.vector.tensor_copy(u_sb[:], psu[:])
                            nc.gpsimd.dma_start(orow, u_sb[:])
                        elif a == 0:
                            nc.scalar.activation(u_sb[:], psu[:], copy_fn)
                            nc.sync.dma_start(orow, u_sb[:])
                        else:  # a == 1: halves on both engines
                            nc.vector.tensor_copy(u_sb[:, :512], psu[:, :512])
                            nc.sync.dma_start(orow[:, :512], u_sb[:, :512])
                            nc.scalar.activation(u_sb[:, 512:], psu[:, 512:],
                                                 copy_fn)
                            nc.gpsimd.dma_start(orow[:, 512:], u_sb[:, 512:])
                # a3: h-major accumulation so h0 completes 2 matmuls early
                psu3 = psup.tile([128, C], f32, tag="psu", name=f"psu_{p}_3")
                for h in range(2):
                    for j in range(KD):
                        nc.tensor.matmul(
                            psu3[:, h * 512: (h + 1) * 512],
                            z_sb[:, j, 3 * 128: 4 * 128],
                            wu_sb[:, j * 1024 + h * 512:
                                  j * 1024 + h * 512 + 512],
                            start=(j == 0), stop=(j == KD - 1),
                            skip_group_check=True,
                        )
                    u3 = upool.tile([128, C // 2], out_dt, tag="u",
                                    name=f"u_last_3{h}")
                    orow3 = out[m, b, 3 * 128: 4 * 128, :]
                    if h == 0:
                        nc.vector.tensor_copy(u3[:], psu3[:, :512])
                        nc.sync.dma_start(orow3[:, :512], u3[:])
                    else:
                        nc.scalar.activation(u3[:], psu3[:, 512:], copy_fn)
                        nc.gpsimd.dma_start(orow3[:, 512:], u3[:])

    nc.compile()
    return nc


def _get_nc():
    if "nc" not in _cache:
        _cache["nc"] = _build()
    return _cache["nc"]


def kernel(x, expert_index, down_w, down_b, up_w):
    global last_results
    import ml_dtypes
    from concourse import bass_utils

    x = np.asarray(x, dtype=np.float32)
    idx = np.asarray(expert_index)
    r = np.arange(M)[:, None]
    wd = np.asarray(down_w, dtype=np.float32)[r, idx]   # [M, B, C, D]
    bd = np.asarray(down_b, dtype=np.float32)[r, idx]   # [M, B, D]
    wu = np.asarray(up_w, dtype=np.float32)[r, idx]     # [M, B, D, C]

    # Pack into SBUF partition-major layouts (see _build comments).
    xt = x.transpose(0, 2, 1).reshape(B, 2, KC // 2, 128, S)
    xt = xt.transpose(0, 1, 3, 2, 4).reshape(B, 2, 128, KC * S // 2)
    wdp = wd.reshape(M, B, KC, 128, D).transpose(0, 1, 3, 2, 4)
    wdp = wdp.reshape(M, B, 128, KC * D)
    wup = wu.reshape(M, B, KD, 128, C).transpose(0, 1, 3, 2, 4)
    wup = wup.reshape(M, B, 128, KD * C)
    bdp = bd.reshape(M, B, KD, 128).transpose(3, 0, 1, 2)  # [128, M, B, KD]

    in_dt = ml_dtypes.bfloat16

    in_maps = []
    for i in range(N_CORES):
        bs = slice(i * B_LOC, (i + 1) * B_LOC)
        # bias rows padded to 576B (see _build): cols 0:16 real, rest zero
        bias_pad = np.zeros((128, 144), dtype=np.float32)
        bias_pad[:, :M * B_LOC * KD] = \
            bdp[:, :, bs, :].reshape(128, M * B_LOC * KD)
        in_maps.append({
            "xtp": np.ascontiguousarray(xt[bs].astype(in_dt)),
            "wdp": np.ascontiguousarray(wdp[:, bs].astype(in_dt)),
            "wup": np.ascontiguousarray(wup[:, bs].astype(in_dt)),
            "bdp": bias_pad,
        })

    nc = _get_nc()
    res = None
    for attempt in range(3):
        try:
            res = bass_utils.run_bass_kernel_spmd(nc, in_maps,
                                                  core_ids=list(range(N_CORES)))
            break
        except Exception:
            # transient NRT_EXEC_UNIT_UNRECOVERABLE device hiccups recover
            # after a short wait; re-raise if persistent
            if attempt == 2:
                raise
            import time
            time.sleep(15)
    last_results = res

    full = np.empty((M, B, S, C), dtype=np.float32)
    for i in range(N_CORES):
        full[:, i * B_LOC:(i + 1) * B_LOC] = np.asarray(
            res.results[i]["out"]).astype(np.float32)
    return full


# revision 14
# speedup vs baseline: 1.0309x; 1.0309x over previous
"""Trainium2 Bass kernel for the AdapterController hard-routing MoE adapter.

Reference computation (per router m in [0,4), batch b in [0,16)):
    e = expert_index[m, b]
    z = x[b] @ down_w[m, e] + down_b[m, e]      # [512, 256]
    z = z * sigmoid(z)                          # swish
    u = z @ up_w[m, e]                          # [512, 1024]
    out[m, b] = u

Strategy: data-parallel over the batch axis (2 batches per core, 8 cores).
The expert gather is part of input sharding: each core receives exactly the
(m, b)-selected weight matrices, packed on the host into the SBUF partition
layout so every DMA is fully contiguous.

On-chip per (m, b) pair:
    zT[d, s] = sum_c Wd[c, d] * xT[c, s]        (16 matmuls N=512, K=128)
    z = silu(zT + bd)                           (ACT engine, PSUM -> SBUF)
    u[s, c] = sum_d zT[d, s].T @ Wu[d, c]       (16 matmuls N=512)

Schedule notes (v2): the matmul stream runs at the warm-PE roofline
(~216 ns per N=512 matmul, 54.6 us total), so the schedule optimizes the
edges:
  - three DMA rings are used: sync (SP HWDGE), scalar (ACT HWDGE), and
    gpsimd (Pool SWDGE).  Head: sync carries wd0 + x(b0) (the stream-start
    critical path; the scalar ring is handicapped ~1.3us by the auto
    act-table load), pool carries bias + wu0 + x(b1), scalar carries
    wd1/wu1.  Steady state: wd(q) on scalar, wu(q) on sync, outs split
    a0/a2->pool, a1->sync, a3->scalar so no ring needs more than ~40% of
    the 360 GB/s DMA-engine pool.
  - the bias transfer is padded to 576B rows (rows below the 512B SDMA
    line-rate minimum degrade into read-modify-write descriptors);
  - the PE HAM clock gate needs ~3.4us of *uninterrupted* matmul activity
    to lift the 1.2 GHz cold throttle, and an idle gap restarts the ramp:
    the warm-up burst of N=128 matmuls is sized so the PE never idles
    between warm-up end and first-data arrival (~11.7us);
  - the first pair runs its down-projection k-outer so both PSUM groups
    chase each arriving x chunk;
  - the last pair reorders its up-projection so a2 completes first and the
    final a3 block is h-major; final copies alternate vector/scalar and the
    final outs drain on three rings in parallel to shorten the tail.
"""

import numpy as np

M, B, S, C, D = 4, 16, 512, 1024, 256
N_CORES = 8
B_LOC = B // N_CORES  # batches per core
KC = C // 128         # 8 c-chunks
KD = D // 128         # 2 d-chunks
NPAIR = M * B_LOC     # 8 (m, b) pairs per core
WARM = 52             # warm-up matmuls (N=128) bridging until first data

_cache = {}
last_results = None  # BassKernelResults of the most recent run (for test.py)


def _build():
    from contextlib import ExitStack

    import concourse.mybir as mybir
    import concourse.tile as tile
    from concourse import bacc
    f32 = mybir.dt.float32
    bf16 = mybir.dt.bfloat16
    mm_dt = bf16
    out_dt = bf16

    nc = bacc.Bacc("TRN2", target_bir_lowering=False, debug=False,
                   num_devices=N_CORES)
    # xtp[b, half][p, k*512 + s] = x[b, s, 128*(4*half + k) + p]
    xtp = nc.dram_tensor("xtp", [B_LOC, 2, 128, KC * S // 2], bf16,
                         kind="ExternalInput").ap()
    # wdp[m, b][p, k*256 + d] = down_w_gathered[m, b, 128k + p, d]
    wdp = nc.dram_tensor("wdp", [M, B_LOC, 128, KC * D], bf16,
                         kind="ExternalInput").ap()
    # bdp[p, (m*B_LOC+b)*2 + j] = down_b_gathered[m, b, 128j + p]; padded to
    # 144 f32 columns so each DMA row is 576B (>= the 512B SDMA line-rate
    # minimum)
    bdp = nc.dram_tensor("bdp", [128, 144], f32, kind="ExternalInput").ap()
    # wup[m, b][p, j*1024 + c] = up_w_gathered[m, b, 128j + p, c]
    wup = nc.dram_tensor("wup", [M, B_LOC, 128, KD * C], bf16,
                         kind="ExternalInput").ap()
    out = nc.dram_tensor("out", [M, B_LOC, S, C], out_dt,
                         kind="ExternalOutput").ap()

    silu = mybir.ActivationFunctionType.Silu
    copy_fn = mybir.ActivationFunctionType.Copy

    with tile.TileContext(nc) as tc, ExitStack() as ctx:
        const = ctx.enter_context(tc.tile_pool(name="const", bufs=1))
        xpool = ctx.enter_context(tc.tile_pool(name="xpool", bufs=4))
        wdpool = ctx.enter_context(tc.tile_pool(name="wdpool", bufs=4))
        wupool = ctx.enter_context(tc.tile_pool(name="wupool", bufs=4))
        zpool = ctx.enter_context(tc.tile_pool(name="zpool", bufs=2))
        upool = ctx.enter_context(tc.tile_pool(name="upool", bufs=12))
        pszp = ctx.enter_context(tc.tile_pool(name="pszp", bufs=2, space="PSUM"))
        psup = ctx.enter_context(tc.tile_pool(name="psup", bufs=3, space="PSUM"))

        bd_sb = const.tile([128, 144], f32)

        # PE warm-up: the HAM clock gate needs ~3.4us of uninterrupted PE
        # activity to lift the 1.2GHz cold throttle, and an idle gap restarts
        # the ramp; the burst is sized to bridge from program start until the
        # first weights/x land (~12us)
        warm_src = const.tile([128, 128], mm_dt)
        nc.gpsimd.memset(warm_src[:], 0)
        warm_ps = pszp.tile([128, 128], f32, tag="psz", name="warm_ps")
        for _ in range(WARM):
            nc.tensor.matmul(warm_ps[:], warm_src[:], warm_src[:],
                             start=True, stop=True)

        # Each DMA ring drains FIFO in trigger order, so per-ring emission
        # order pins byte-arrival order. (No dep-chaining between DMAs: a dep
        # on a DMA instruction waits for its data semaphore, which would
        # serialize transfer-after-completion.)
        xh = {b: [xpool.tile([128, KC * S // 2], mm_dt, tag="xt",
                             name=f"xt_{b}_{h}") for h in range(2)]
              for b in range(B_LOC)}
        wd_t = {p: wdpool.tile([128, KC * D], mm_dt, tag="wd",
                               name=f"wd{p}")
                for p in range(2)}
        wu_t = {p: wupool.tile([128, KD * C], mm_dt, tag="wu", name=f"wu{p}")
                for p in range(2)}

        # Head fill.  HW model: each HWDGE engine (sync=SP, scalar=ACT) owns
        # 4 hardware queues; every un-blocked dma_start costs ~0.6us on its
        # sequencer AND ~0.63us on the single shared HWDGE descriptor
        # generator, then all ACTIVE queues share the 16 DMA engines
        # (~360 GB/s) round-robin.  Transfers on one engine do NOT
        # serialize; arrival order is set by cfg-stagger + fair sharing.
        # So: few transfers, critical ones (x00 on sync, wd0 on scalar)
        # first into the generator; everything later paced behind blocking
        # instructions so it can't steal head bandwidth.
        # The framework hoists the first act-table load (~1.3us) to the
        # front of the scalar stream, delaying scalar's first cfg to ~8.8us;
        # so the stream-critical x00+wd0 (and x01) ride SYNC, and scalar
        # only carries bias+wu0 whose deadlines are ~4us later.
        nc.sync.dma_start(xh[0][0][:], xtp[0, 0])    # x00
        nc.sync.dma_start(wd_t[0][:], wdp[0, 0])     # wd0
        nc.sync.dma_start(xh[0][1][:], xtp[0, 1])    # x01
        nc.scalar.dma_start(bd_sb[:], bdp[:])        # bias (576B rows)
        nc.scalar.dma_start(wu_t[0][:, :1024], wup[0, 0][:, :1024])  # wu0-j0
        nc.scalar.dma_start(wu_t[0][:, 1024:], wup[0, 0][:, 1024:])  # wu0-j1
        # wd1/wu1/wu2 are paced: a tiny vector pre-write into each tile
        # completes only when its timing source (bias / x01 / wu0-j1 DMA)
        # lands, and the WAW dep holds the big transfer back until then --
        # so they cannot steal head bandwidth from the critical bytes.
        # Their cfgs sit on the sync sequencer, which runs no compute.
        wu_t[2] = wupool.tile([128, KD * C], mm_dt, tag="wu", name="wu2")
        nc.vector.tensor_copy(wd_t[1][:, 0:4], bd_sb[:, 0:4])
        nc.vector.tensor_copy(wu_t[1][:, 0:4], xh[0][1][:, 0:4])
        nc.vector.tensor_copy(wu_t[2][:, 0:4], wu_t[0][:, 1024:1028])
        nc.sync.dma_start(wd_t[1][:], wdp[1, 0])   # wd1 (paced by bias)
        nc.sync.dma_start(wu_t[1][:], wup[1, 0])   # wu1 (paced by x01)
        nc.sync.dma_start(wu_t[2][:], wup[2 % M, 2 // M])  # wu2 (by wu0-j1)
        # x(b1) rides the pool ring, held back by a WAW dep on its first
        # columns (written from pair 0's first out tile) so it can't start
        # before ~T0+4us no matter how the scheduler orders the pool stream.

        for p in range(NPAIR):
            m, b = p % M, p // M
            q = p + 2
            if q < NPAIR:
                mq, bq = q % M, q // M
                wd_t[q] = wdpool.tile([128, KC * D], mm_dt, tag="wd",
                                      name=f"wd{q}")
            q3 = p + 3
            if 3 <= q3 < NPAIR:
                wu_t[q3] = wupool.tile([128, KD * C], mm_dt, tag="wu",
                                       name=f"wu{q3}")

            wd_sb, wu_sb, xb = wd_t[p], wu_t[p], xh[b]
            z_sb = zpool.tile([128, KD, S], mm_dt)
            if p == 0:
                # k-outer for the first pair: both PSUM groups consume each
                # arriving x chunk, halving the early x-consumption rate so
                # the matmuls bridge the x(b0) half-1 delivery
                psz_j = [pszp.tile([128, S], f32, tag="psz", name=f"psz{j}")
                         for j in range(KD)]
                for k in range(KC):
                    for j in range(KD):
                        nc.tensor.matmul(
                            psz_j[j][:],
                            wd_sb[:, k * 256 + j * 128:
                                  k * 256 + j * 128 + 128],
                            xb[k // 4][:, (k % 4) * S: (k % 4 + 1) * S],
                            start=(k == 0), stop=(k == KC - 1),
                        )
                for j in range(KD):
                    col = (m * B_LOC + b) * KD + j
                    nc.scalar.activation(z_sb[:, j, :], psz_j[j][:], silu,
                                         bias=bd_sb[:, col: col + 1])
                    if j == 0 and q < NPAIR:
                        # wd(q) cfg is paced behind silu j0 on the scalar
                        # sequencer: it can't fire before ~T0+2us, keeping
                        # the head queues clear for the critical bytes
                        nc.scalar.dma_start(wd_t[q][:], wdp[q % M, q // M])
            else:
                for j in range(KD):
                    psz = pszp.tile([128, S], f32)
                    for k in range(KC):
                        nc.tensor.matmul(
                            psz[:],
                            wd_sb[:, k * 256 + j * 128:
                                  k * 256 + j * 128 + 128],
                            xb[k // 4][:, (k % 4) * S: (k % 4 + 1) * S],
                            start=(k == 0), stop=(k == KC - 1),
                        )
                    col = (m * B_LOC + b) * KD + j
                    nc.scalar.activation(z_sb[:, j, :], psz[:], silu,
                                         bias=bd_sb[:, col: col + 1])
                    if j == 0 and q < NPAIR:
                        nc.scalar.dma_start(wd_t[q][:], wdp[q % M, q // M])

            # one 2-bank PSUM tile per a holds a full [128, 1024] u row;
            # groups are interleaved so j=1 matmuls trail the j=1 silu by a
            # couple of matmul slots (no PE stall).
            last = p == NPAIR - 1
            if not last:
                order = ((0, 0), (0, 1), (1, 0), (1, 1),
                         (0, 2), (0, 3), (1, 2), (1, 3))
                psu_by_a = {}
                for j, a in order:
                    if j == 0:
                        psu_by_a[a] = psup.tile([128, C], f32, tag="psu",
                                                name=f"psu_{p}_{a}")
                    psu = psu_by_a[a]
                    for h in range(2):
                        nc.tensor.matmul(
                            psu[:, h * 512: (h + 1) * 512],
                            z_sb[:, j, a * 128: (a + 1) * 128],
                            wu_sb[:, j * 1024 + h * 512:
                                  j * 1024 + h * 512 + 512],
                            start=(j == 0), stop=(j == KD - 1),
                            skip_group_check=True,
                        )
                    if j == KD - 1:
                        u_sb = upool.tile([128, C], out_dt, tag="u")
                        orow = out[m, b, a * 128:(a + 1) * 128, :]
                        if a % 2 == 0:
                            nc.vector.tensor_copy(u_sb[:], psu[:])
                        else:
                            nc.scalar.activation(u_sb[:], psu[:], copy_fn)
                        # outs: a0/a2 on pool queues, a1 on sync, a3 on
                        # scalar -- spreads output traffic so weight
                        # transfers never queue behind it
                        if a in (0, 2):
                            nc.gpsimd.dma_start(orow, u_sb[:])
                        elif a == 1:
                            nc.sync.dma_start(orow, u_sb[:])
                            if 3 <= p + 3 < NPAIR:
                                # wu(p+3) cfg paced behind the a1 out
                                # trigger on the sync sequencer (~T0+5+6.8p)
                                nc.sync.dma_start(wu_t[p + 3][:],
                                                 wup[(p + 3) % M,
                                                     (p + 3) // M])
                        else:
                            nc.scalar.dma_start(orow, u_sb[:])
                        if p == 0 and a == 0:
                            # x(b1): a WAW dep on its first columns (written
                            # from pair 0's first out tile) holds the
                            # transfer back until ~T0+4us regardless of how
                            # the scheduler orders the pool stream
                            nc.gpsimd.tensor_copy(xh[1][0][:, 0:4],
                                                  u_sb[:, 0:4])
                            nc.gpsimd.dma_start(xh[1][0][:], xtp[1, 0])
                            nc.gpsimd.tensor_copy(xh[1][1][:, 0:4],
                                                  u_sb[:, 4:8])
                            nc.gpsimd.dma_start(xh[1][1][:], xtp[1, 1])
            else:
                # Final pair: a2 completes first, then a0, a1; a3 runs
                # h-major so its first half can be copied + DMA'd while the
                # second half's matmuls run.  Copies split vector/scalar so
                # neither engine's chain extends past the last matmul by
                # more than one half-copy; out cfgs spread over sync (3) and
                # pool (3) so no sequencer serializes more than ~3 cfgs.
                order = ((0, 2), (0, 0), (1, 2), (1, 0), (0, 1), (1, 1))
                psu_by_a = {}
                u_by_a = {}
                for j, a in order:
                    if j == 0:
                        psu_by_a[a] = psup.tile([128, C], f32, tag="psu",
                                                name=f"psu_{p}_{a}")
                    psu = psu_by_a[a]
                    for h in range(2):
                        nc.tensor.matmul(
                            psu[:, h * 512: (h + 1) * 512],
                            z_sb[:, j, a * 128: (a + 1) * 128],
                            wu_sb[:, j * 1024 + h * 512:
                                  j * 1024 + h * 512 + 512],
                            start=(j == 0), stop=(j == KD - 1),
                            skip_group_check=True,
                        )
                    if j == KD - 1:
                        u_sb = upool.tile([128, C], out_dt, tag="u",
                                          name=f"u_last_{a}")
                        u_by_a[a] = u_sb
                        orow = out[m, b, a * 128:(a + 1) * 128, :]
                        if a == 2:
                            nc.vector.tensor_copy(u_sb[:], psu[:])
                            nc.gpsimd.dma_start(orow, u_sb[:])
                        elif a == 0:
                            nc.scalar.activation(u_sb[:], psu[:], copy_fn)
                            nc.sync.dma_start(orow, u_sb[:])
                        else:  # a == 1: halves on both engines
                            nc.vector.tensor_copy(u_sb[:, :512], psu[:, :512])
                            nc.sync.dma_start(orow[:, :512], u_sb[:, :512])
                            nc.scalar.activation(u_sb[:, 512:], psu[:, 512:],
                                                 copy_fn)
                            nc.gpsimd.dma_start(orow[:, 512:], u_sb[:, 512:])
                # a3: h-major accumulation so h0 completes 2 matmuls early
                psu3 = psup.tile([128, C], f32, tag="psu", name=f"psu_{p}_3")
                for h in range(2):
                    for j in range(KD):
                        nc.tensor.matmul(
                            psu3[:, h * 512: (h + 1) * 512],
                            z_sb[:, j, 3 * 128: 4 * 128],
                            wu_sb[:, j * 1024 + h * 512:
                                  j * 1024 + h * 512 + 512],
                            start=(j == 0), stop=(j == KD - 1),
                            skip_group_check=True,
                        )
                    u3 = upool.tile([128, C // 2], out_dt, tag="u",
                                    name=f"u_last_3{h}")
                    orow3 = out[m, b, 3 * 128: 4 * 128, :]
                    if h == 0:
                        nc.vector.tensor_copy(u3[:], psu3[:, :512])
                        nc.sync.dma_start(orow3[:, :512], u3[:])
                    else:
                        nc.scalar.activation(u3[:], psu3[:, 512:], copy_fn)
                        nc.gpsimd.dma_start(orow3[:, 512:], u3[:])

    nc.compile()
    return nc


def _get_nc():
    if "nc" not in _cache:
        _cache["nc"] = _build()
    return _cache["nc"]


def kernel(x, expert_index, down_w, down_b, up_w):
    global last_results
    import ml_dtypes
    from concourse import bass_utils

    x = np.asarray(x, dtype=np.float32)
    idx = np.asarray(expert_index)
    r = np.arange(M)[:, None]
    wd = np.asarray(down_w, dtype=np.float32)[r, idx]   # [M, B, C, D]
    bd = np.asarray(down_b, dtype=np.float32)[r, idx]   # [M, B, D]
    wu = np.asarray(up_w, dtype=np.float32)[r, idx]     # [M, B, D, C]

    # Pack into SBUF partition-major layouts (see _build comments).
    xt = x.transpose(0, 2, 1).reshape(B, 2, KC // 2, 128, S)
    xt = xt.transpose(0, 1, 3, 2, 4).reshape(B, 2, 128, KC * S // 2)
    wdp = wd.reshape(M, B, KC, 128, D).transpose(0, 1, 3, 2, 4)
    wdp = wdp.reshape(M, B, 128, KC * D)
    wup = wu.reshape(M, B, KD, 128, C).transpose(0, 1, 3, 2, 4)
    wup = wup.reshape(M, B, 128, KD * C)
    bdp = bd.reshape(M, B, KD, 128).transpose(3, 0, 1, 2)  # [128, M, B, KD]

    in_dt = ml_dtypes.bfloat16

    in_maps = []
    for i in range(N_CORES):
        bs = slice(i * B_LOC, (i + 1) * B_LOC)
        # bias rows padded to 576B (see _build): cols 0:16 real, rest zero
        bias_pad = np.zeros((128, 144), dtype=np.float32)
        bias_pad[:, :M * B_LOC * KD] = \
            bdp[:, :, bs, :].reshape(128, M * B_LOC * KD)
        in_maps.append({
            "xtp": np.ascontiguousarray(xt[bs].astype(in_dt)),
            "wdp": np.ascontiguousarray(wdp[:, bs].astype(in_dt)),
            "wup": np.ascontiguousarray(wup[:, bs].astype(in_dt)),
            "bdp": bias_pad,
        })

    nc = _get_nc()
    res = None
    for attempt in range(3):
        try:
            res = bass_utils.run_bass_kernel_spmd(nc, in_maps,
                                                  core_ids=list(range(N_CORES)))
            break
        except Exception:
            # transient NRT_EXEC_UNIT_UNRECOVERABLE device hiccups recover
            # after a short wait; re-raise if persistent
            if attempt == 2:
                raise
            import time
            time.sleep(15)
    last_results = res

    full = np.empty((M, B, S, C), dtype=np.float32)
    for i in range(N_CORES):
        full[:, i * B_LOC:(i + 1) * B_LOC] = np.asarray(
            res.results[i]["out"]).astype(np.float32)
    return full
